# revision 1
# baseline (speedup 1.0000x reference)
"""ConvLSTM cell (B=32, C_IN=32, HC=64, H=W=64, K=3) on 8 trn2 NeuronCores.

Strategy: data-parallel over batch (4 images per core), weights replicated.
x (32ch) and h (64ch) are concatenated on host into one bf16 tensor; each
16-row block loads a contiguous [96, 18, 64] tile (1-row halo each side).
The fused conv (-> 256 gate channels) is 9 shifted matmuls per 128-channel
chunk accumulating in PSUM; image-border taps use row/column-restricted
access patterns instead of a zero-padded layout (the center tap runs first
with start=True so every PSUM element is initialized).
Gate chunks: chunk0 = [f, i], chunk1 = [o, g]; the LSTM elementwise math
runs mostly full-width, with one gpsimd partition-shift copy and a single
merged [h_new | c_new] output DMA per block.
"""

import os
import sys

import numpy as np

if "/opt/trn_rl_repo" not in sys.path:
    sys.path.insert(0, "/opt/trn_rl_repo")

import ml_dtypes

BF16 = ml_dtypes.bfloat16

B, C_IN, HC, H, W, K = 32, 32, 64, 64, 64, 3
N_CORES = 8
B_LOC = B // N_CORES  # 4 images per core
CTOT = C_IN + HC  # 96 combined input channels
RPB = 16  # output rows per block
NBLK = H // RPB
SUB_ROWS = 8  # rows per matmul (512 px = one PSUM bank)
SUB_PX = SUB_ROWS * W
# tap order: center (dy=1,dx=1) first so start=True covers every element
TAP_ORDER = [4, 0, 1, 2, 3, 5, 6, 7, 8]

_CACHE: dict = {}


def _build_program():
    import concourse.bacc as bacc
    import concourse.mybir as mybir
    import concourse.tile as tile

    nc = bacc.Bacc("TRN2", target_bir_lowering=False, debug=False)
    f32 = mybir.dt.float32
    bf16 = mybir.dt.bfloat16
    AF = mybir.ActivationFunctionType

    xh_d = nc.dram_tensor("xh", [B_LOC, CTOT, H, W], bf16, kind="ExternalInput").ap()
    c_d = nc.dram_tensor("c", [B_LOC, HC, H, W], f32, kind="ExternalInput").ap()
    w_d = nc.dram_tensor("w", [CTOT, 9 * 4 * HC], bf16, kind="ExternalInput").ap()
    b_d = nc.dram_tensor("bias", [128, 2], f32, kind="ExternalInput").ap()
    # out[:, 0] = h_new, out[:, 1] = c_new
    out_d = nc.dram_tensor(
        "out", [B_LOC, 2, HC, H, W], f32, kind="ExternalOutput"
    ).ap()

    with tile.TileContext(nc) as tc:
        with (
            tc.tile_pool(name="const", bufs=1) as constp,
            tc.tile_pool(name="pt", bufs=6) as ptp,
            tc.tile_pool(name="psum0", bufs=2, space="PSUM") as pp0,
            tc.tile_pool(name="psum1", bufs=2, space="PSUM") as pp1,
            tc.tile_pool(name="work", bufs=3) as sp,
        ):
            w_sb = constp.tile([CTOT, 9 * 4 * HC], bf16)
            nc.scalar.dma_start(w_sb[:], w_d)
            b_sb = constp.tile([128, 2], f32)
            nc.scalar.dma_start(b_sb[:], b_d)

            def stage_b(st):
                # deferred tail of a block: tanh(c_new), h_new, output DMA
                b_, y0_, rpb_, so_, chn_, i_ = st
                tch = sp.tile([64, rpb_ * W], f32, tag="tch", name=f"tch{i_}")
                nc.scalar.activation(tch[:], chn_[64:128, :], AF.Tanh)
                nc.vector.tensor_mul(chn_[0:64, :], so_[:], tch[:])
                nc.sync.dma_start(
                    out_d[b_, :, :, y0_ : y0_ + rpb_, :].rearrange(
                        "t c y x -> (t c) y x"
                    ),
                    chn_[:].rearrange("p (y x) -> p y x", x=W),
                )

            # PE prewarm: ~12 dummy matmuls on zeroed tiles so the HAM
            # clock gate opens before the first real matmul arrives
            dw = constp.tile([CTOT, 128], bf16)
            nc.gpsimd.memset(dw[:], 0.0)
            drh = constp.tile([CTOT, SUB_PX], bf16)
            nc.gpsimd.memset(drh[:], 0.0)
            pwp = pp0.tile([128, RPB * W], f32, tag="P0", name="pw")
            for _ in range(12):
                nc.tensor.matmul(
                    pwp[:, 0:SUB_PX], dw[:], drh[:], start=True, stop=True
                )

            pending = None
            bi = 0
            for b in range(B_LOC):
                rpb = 8 if b == B_LOC - 1 else RPB
                nblk = H // rpb
                for blk in range(nblk):
                    y0 = blk * rpb
                    nrows = rpb + 2
                    blk_px = rpb * W
                    # contiguous input tile; local row L = image row y0-1+L
                    pt = ptp.tile(
                        [CTOT, nrows * W], bf16, tag="ptb", name=f"ptb{bi}"
                    )
                    pt3 = pt[:].rearrange("c (y x) -> c y x", x=W)
                    gs = max(0, y0 - 1)
                    ge = min(H, y0 + rpb + 1)
                    ls = gs - (y0 - 1)
                    le = ge - (y0 - 1)
                    nc.sync.dma_start(pt3[:, ls:le, :], xh_d[b, :, gs:ge, :])

                    P0 = pp0.tile([128, blk_px], f32, tag="P0", name=f"P0_{bi}")
                    P1 = pp1.tile([128, blk_px], f32, tag="P1", name=f"P1_{bi}")
                    for chunk, P in ((0, P0), (1, P1)):
                        P3 = P[:].rearrange("c (y x) -> c y x", x=W)
                        for ti, off in enumerate(TAP_ORDER):
                            dy, dx = off // 3, off % 3
                            lo = off * 256 + chunk * 128
                            lhsT = w_sb[:, lo : lo + 128]
                            # border-valid output ranges for this tap
                            r_lo = 1 if (blk == 0 and dy == 0) else 0
                            r_hi = (
                                rpb - 2
                                if (blk == nblk - 1 and dy == 2)
                                else rpb - 1
                            )
                            cout0, ncols = ((1, 63), (0, 64), (0, 63))[dx]
                            cin0 = cout0 + dx - 1
                            for sub in range(rpb // SUB_ROWS):
                                r0 = max(sub * SUB_ROWS, r_lo)
                                r1 = min(sub * SUB_ROWS + SUB_ROWS - 1, r_hi)
                                rhs = pt3[
                                    :, r0 + dy : r1 + 1 + dy, cin0 : cin0 + ncols
                                ]
                                nc.tensor.matmul(
                                    P3[:, r0 : r1 + 1, cout0 : cout0 + ncols],
                                    lhsT,
                                    rhs,
                                    start=(ti == 0),
                                    stop=(ti == 8),
                                )

                    # elementwise LSTM math for this block
                    # P0 = [f | i], P1 = [o | g] (by 64-partition halves)
                    s_fi = sp.tile([128, blk_px], f32, tag="sfi", name=f"sfi{bi}")
                    nc.scalar.activation(
                        s_fi[:], P0[:], AF.Sigmoid, bias=b_sb[:, 0:1]
                    )
                    so = sp.tile([64, blk_px], f32, tag="so", name=f"so{bi}")
                    nc.scalar.activation(
                        so[:], P1[0:64, :], AF.Sigmoid, bias=b_sb[0:64, 1:2]
                    )
                    cg = sp.tile([128, blk_px], f32, tag="cg", name=f"cg{bi}")
                    nc.scalar.activation(
                        cg[64:128, :], P1[64:128, :], AF.Tanh, bias=b_sb[64:128, 1:2]
                    )
                    nc.gpsimd.dma_start(
                        cg[0:64, :].rearrange("c (y x) -> c y x", x=W),
                        c_d[b, :, y0 : y0 + rpb, :],
                    )
                    # prd = [f*c | i*g]
                    prd = sp.tile([128, blk_px], f32, tag="prd", name=f"prd{bi}")
                    nc.vector.tensor_mul(prd[:], s_fi[:], cg[:])
                    igc = sp.tile([64, blk_px], f32, tag="igc", name=f"igc{bi}")
                    nc.vector.tensor_copy(igc[:], prd[64:128, :])
                    # chn = [h_new | c_new] merged output tile
                    chn = sp.tile([128, blk_px], f32, tag="chn", name=f"chn{bi}")
                    nc.vector.tensor_add(chn[64:128, :], prd[0:64, :], igc[:])
                    if pending is not None:
                        stage_b(pending)
                    pending = (b, y0, rpb, so, chn, bi)
                    bi += 1
            stage_b(pending)

    nc.compile()
    return nc


def get_program():
    if "nc" not in _CACHE:
        _CACHE["nc"] = _build_program()
    return _CACHE["nc"]


def _prep_host(inputs):
    """Pack weights/biases; convert x/h to bf16; build per-core input maps."""
    x = np.asarray(inputs["x"], np.float32)
    h = np.asarray(inputs["hidden_state"], np.float32)
    c = np.ascontiguousarray(np.asarray(inputs["cell_state"], np.float32))

    # gate column order [f, i, o, g] -> chunk0=[f,i], chunk1=[o,g]
    gx = [inputs["w_xf"], inputs["w_xi"], inputs["w_xo"], inputs["w_xg"]]
    gh = [inputs["w_hf"], inputs["w_hi"], inputs["w_ho"], inputs["w_hg"]]
    wx = np.stack([np.asarray(a, np.float32) for a in gx])  # [4, HC, C_IN, 3, 3]
    wh = np.stack([np.asarray(a, np.float32) for a in gh])  # [4, HC, HC, 3, 3]
    # -> [c, dy, dx, gate, o] -> [c, 9, 256]
    wxc = np.transpose(wx, (2, 3, 4, 0, 1)).reshape(C_IN, 9, 4 * HC)
    whc = np.transpose(wh, (2, 3, 4, 0, 1)).reshape(HC, 9, 4 * HC)
    wcat = np.concatenate([wxc, whc], 0).reshape(CTOT, 9 * 4 * HC).astype(BF16)

    bf = np.asarray(inputs["b_xf"], np.float32) + np.asarray(inputs["b_hf"], np.float32)
    bi = np.asarray(inputs["b_xi"], np.float32) + np.asarray(inputs["b_hi"], np.float32)
    bo = np.asarray(inputs["b_xo"], np.float32) + np.asarray(inputs["b_ho"], np.float32)
    bg = np.asarray(inputs["b_xg"], np.float32) + np.asarray(inputs["b_hg"], np.float32)
    bias = np.stack(
        [np.concatenate([bf, bi]), np.concatenate([bo, bg])], axis=1
    ).astype(np.float32)  # [128, 2]

    xh = np.concatenate([x, h], axis=1).astype(BF16)  # [B, 96, H, W]

    in_maps = []
    for i in range(N_CORES):
        s = slice(i * B_LOC, (i + 1) * B_LOC)
        in_maps.append(
            {
                "xh": xh[s],
                "c": c[s],
                "w": wcat,
                "bias": bias,
            }
        )
    return in_maps


def run(inputs, trace=False, trace_kwargs=None):
    from concourse.bass_utils import run_bass_kernel_spmd

    nc = get_program()
    in_maps = _prep_host(inputs)
    res = run_bass_kernel_spmd(
        nc,
        in_maps,
        list(range(N_CORES)),
        trace=trace,
        **(trace_kwargs or {}),
    )
    h_new = np.concatenate([r["out"][:, 0] for r in res.results], 0).astype(
        np.float32
    )
    c_new = np.concatenate([r["out"][:, 1] for r in res.results], 0).astype(
        np.float32
    )
    return (h_new, c_new), res


def kernel(**inputs):
    (h_new, c_new), _ = run(inputs, trace=False)
    return (h_new, c_new)

